# revision 18
# baseline (speedup 1.0000x reference)
"""Trainium2 kernel for BandDecimate: scipy.signal.decimate(x, q=4, n=8,
ftype='iir', zero_phase=True) on x of shape (32, 16, 65536).

Method: filtfilt with the order-8 Chebyshev-I filter is re-expressed as a
single symmetric FIR g = corr(h, h) (h = 512-tap truncated impulse
response), truncated to its central 256 taps (tail l2 = 1e-3), applied to
the odd-extended, steady-state-padded signal, fused with the decimation
by 4 via a 4-phase polyphase decomposition.  Phases are PAIRED along the
PE contraction dim (2 phases x 64 samples per 128-row column), so each
128-output block needs only 6 block-Toeplitz matmuls (2 phase-pairs x 3
column offsets).  Data, weights and outputs move in bf16 (PSUM
accumulates fp32).  The right edge (last 128 decimated outputs), where
the backward-pass initial condition differs from the symmetric-FIR
approximation, is computed exactly by a dense precomputed 128x1024
linear map (8 extra matmuls).  512 independent series are sharded
64-per-core across 8 cores.
"""
import os
import sys

import numpy as np

sys.path.insert(0, "/opt/trn_rl_repo")

# ---------------------------------------------------------------- constants
Q = 4
N_ORDER = 8
RP = 0.05
T = 65536
EDGE = 27
L0 = T + 2 * EDGE          # 65590
P = 512                    # truncated IIR impulse response length
NPH = 4                    # polyphase phases
GOFF = 384                 # central-window offset into the 1024-tap G
KTAP = 256                 # total taps kept (64 per phase)
LPAD = 100                 # left steady-state pad of u (484 - GOFF)
ULEN = 66048               # 256 * 258; RPAD = ULEN - LPAD - L0 = 358
NOUT = T // Q              # 16384 outputs per series
M2 = 129                   # V columns (pairs of 64-sample half-blocks)
S = 64                     # series per core
NCORES = 8
W_EDGE = 1024              # edge window length (8 * 128)
KK = W_EDGE // 128         # 8
NW = 6                     # main weight matrices (2 phase-pairs x 3 offsets)
BLK = 8                    # output columns per matmul (free = BLK*S = 512)
# mostly-big transfers (small ones waste ~65ns/packet of DMA-engine time);
# small first chunk = early PE start, small last chunks = short tail
CHUNK_COLS = (16, 32, 32, 24, 16, 8)   # output n-cols per chunk
CHUNK_J0 = (0, 16, 48, 80, 104, 120)
OUT_GROUPS = ((0,), (1,), (2,), (3,), (4,), (5,))  # chunks per out tensor
NDUMMY_PRE = 8             # PE warm-up matmuls before chunk0 lands
NDUMMY_POST = 12           # PE warm-up matmuls in the post-edge DMA gap

DT_MM = os.environ.get("BASS_MM_DTYPE", "bfloat16")


# ------------------------------------------------------------- filter design
def _design():
    eps = np.sqrt(10.0 ** (0.1 * RP) - 1.0)
    mu = np.arcsinh(1.0 / eps) / N_ORDER
    k = np.arange(1, N_ORDER + 1)
    theta = np.pi * (2 * k - 1) / (2 * N_ORDER)
    p = -np.sinh(mu) * np.sin(theta) + 1j * np.cosh(mu) * np.cos(theta)
    g = np.prod(-p).real
    if N_ORDER % 2 == 0:
        g /= np.sqrt(1.0 + eps**2)
    fs = 2.0
    warped = 2.0 * fs * np.tan(np.pi * (0.8 / Q) / fs)
    p = p * warped
    g = g * warped**N_ORDER
    fs2 = 2.0 * fs
    pd = (fs2 + p) / (fs2 - p)
    zd = -np.ones(N_ORDER)
    gd = g * np.real(1.0 / np.prod(fs2 - p))
    b = np.real(gd * np.poly(zd))
    a = np.real(np.poly(pd))
    n = len(a)
    comp = np.zeros((n - 1, n - 1))
    comp[0, :] = -a[1:] / a[0]
    comp[1:, :-1] = np.eye(n - 2)
    IminusA = np.eye(n - 1) - comp.T
    B = b[1:] - a[1:] * b[0]
    zi = np.linalg.solve(IminusA, B)
    return b, a, zi


def _lfilter(b, a, x, zi):
    """Direct-form II transposed; x: (T, M) float64."""
    z = zi.copy()
    y = np.empty_like(x)
    for t in range(x.shape[0]):
        xt = x[t]
        yt = b[0] * xt + z[0]
        y[t] = yt
        z = np.concatenate([z[1:], np.zeros_like(z[:1])], axis=0) \
            + b[1:, None] * xt[None, :] - a[1:, None] * yt[None, :]
    return y


def _build_weights():
    """Returns (W6 [128, 6, 128], W_edge [128, KK, 128]) float64."""
    b, a, zi = _design()
    x = np.zeros((P, 1))
    x[0, 0] = 1.0
    h = _lfilter(b, a, x, np.zeros((N_ORDER, 1)))[:, 0]
    g = np.correlate(h, h, mode="full")            # 1023 taps
    Gfull = np.zeros(1024)
    Gfull[: 2 * P - 1] = g
    G = Gfull[GOFF:GOFF + KTAP]                    # central 256 taps

    # main matrices: lhsT[64*r2 + a, 3*pp + delta, b] = G_{2pp+r2}[64d + a - b]
    aa = np.arange(64)[:, None]
    bb = np.arange(128)[None, :]
    W6 = np.zeros((128, NW, 128))
    for pp in range(2):
        for r2 in range(2):
            r = 2 * pp + r2
            Gr = G[r::NPH]                         # 64 taps
            for delta in range(3):
                tau = 64 * delta + aa - bb
                valid = (tau >= 0) & (tau < 64)
                W6[64 * r2:64 * r2 + 64, 3 * pp + delta, :] = \
                    np.where(valid, Gr[np.clip(tau, 0, 63)], 0.0)

    # edge matrix: exact last-128 outputs as linear map of last 1024 ext samples
    t_idx = np.arange(W_EDGE)
    w_idx = np.arange(W_EDGE)
    d_idx = t_idx[:, None] - w_idx[None, :]
    hmat = np.where((d_idx >= 0) & (d_idx < P), h[np.clip(d_idx, 0, P - 1)], 0.0)
    y1 = hmat                                      # [t, w] forward FIR basis
    y1_rev = y1[::-1]
    z0 = zi[:, None] * y1_rev[0][None, :]
    y2 = _lfilter(b, a, y1_rev, z0)[::-1]
    S0 = L0 - W_EDGE
    js = np.arange(NOUT - 128, NOUT)
    ts = EDGE + 4 * js - S0
    M_edge = y2[ts, :]                             # [128, 1024]
    W_edge = M_edge.reshape(128, KK, 128).transpose(2, 1, 0)  # [q, kk, j]
    return W6, W_edge


_CACHE = {}


def _prep_static():
    if "w" not in _CACHE:
        W6, W_edge = _build_weights()
        _CACHE["w"] = (np.ascontiguousarray(W6, np.float32),
                       np.ascontiguousarray(W_edge, np.float32))
    return _CACHE["w"]


# ------------------------------------------------------------- bass program
def _make_tile_context_cls():
    from concourse.tile import TileContext
    from concourse.vector_clock import ScopedClock, VectorClock

    class SplitDrainTileContext(TileContext):
        """This walrus build allows very few attached sync-waits per
        instruction; the stock kernel-tail drain carries one wait per DMA
        lane/engine and gets rejected.  Split it into one drain per proc,
        each with a single wait."""

        def _drain_and_barrier(self, tick_clock, wait_clock):
            gc = tick_clock.global_clock
            n = len(gc)
            for proc in range(n):
                if gc[proc] == 0:
                    continue
                vec = [0] * n
                vec[proc] = gc[proc]
                d = self.nc.sync.drain()
                wait_clock.add_sem_waits(d.ins, ScopedClock({None: VectorClock(vec)}))
            self.nc.sync.drain()
            self.nc.all_engine_barrier()
            assert self.sems is not None
            popped = self.nc._tile_sem_poison_stack.pop()
            assert popped is self._sem_poison
            self.nc.clear_and_free_semaphores(list(self.sems.allocated().values()))
            self.nc.all_engine_barrier()

    return SplitDrainTileContext


def _build_nc():
    import concourse.bass as bass
    import concourse.mybir as mybir
    TileContext = _make_tile_context_cls()

    dt_mm = getattr(mybir.dt, DT_MM)
    f32 = mybir.dt.float32

    # cedge layout along free dim: [wedge KK*128 | etail KK*64]
    CE = KK * 128 + KK * S                         # 1536
    OFF_ET = KK * 128

    nc = bass.Bass(target_bir_lowering=False)
    # v[z, m2, c]: z = 64*r2 + a, c = 128*pp + 64*par + s
    v_d = nc.declare_dram_parameter("v", [128, M2, 256], dt_mm, isOutput=False)
    w_d = nc.declare_dram_parameter("wmain", [128, NW * 128], dt_mm, isOutput=False)
    e_d = nc.declare_dram_parameter("cedge", [128, CE], dt_mm, isOutput=False)
    out_ws = [sum(CHUNK_COLS[c] for c in grp) for grp in OUT_GROUPS]
    out_ds = [nc.declare_dram_parameter(f"out{i}", [128, w, S], dt_mm, isOutput=True)
              for i, w in enumerate(out_ws)]

    # Walrus (this version) allows at most ONE attached sync-wait per
    # instruction, so the structure below arranges that every instruction
    # has at most one un-observed dependency:
    #  - chunk/osb pools are deep enough to never recycle a slot
    #  - ps3/eps are single-buffered; tiny "gate" matmuls absorb psum
    #    slot-release waits so chain-start matmuls wait only on their DMA
    with TileContext(nc) as tc:
        with tc.tile_pool(name="const", bufs=1) as cpool, \
             tc.tile_pool(name="vchunk", bufs=len(CHUNK_COLS)) as vpool, \
             tc.tile_pool(name="osb", bufs=len(OUT_GROUPS)) as opool, \
             tc.tile_pool(name="psum", bufs=1, space="PSUM") as ppool:

            ws = cpool.tile([128, NW * 128], dt_mm, tag="ws")
            ce = cpool.tile([128, CE], dt_mm, tag="ce")
            edge_sb = cpool.tile([128, S], f32, tag="edge")

            # DMA issue order on the sync hw queue = transfer order:
            # weights first (unblocks PE), then chunk0, cedge (needed by the
            # edge chain slotted into the pass0->pass1 gap), then the rest.
            nc.sync.dma_start(out=ws[:], in_=w_d[:])
            chunks = []
            for p, (j0, w) in enumerate(zip(CHUNK_J0, CHUNK_COLS)):
                chunk = vpool.tile([128, w + 1, 256], dt_mm, tag="chunk")
                nc.sync.dma_start(out=chunk[:], in_=v_d[:, j0:j0 + w + 1, :])
                chunks.append(chunk)
                if p == 0:
                    nc.sync.dma_start(out=ce[:], in_=e_d[:])

            def wm(pp, delta):
                k = 3 * pp + delta
                return ws[:, k * 128:(k + 1) * 128]

            def we(kk):
                return ce[:, kk * 128:(kk + 1) * 128]

            def et(kk):
                return ce[:, OFF_ET + kk * S: OFF_ET + (kk + 1) * S]

            eps = ppool.tile([128, S], f32, tag="eps", bufs=1)

            def ps_tile(i):
                return ppool.tile([128, BLK, S], f32, tag=f"ps{i}",
                                  name=f"ps{i}", bufs=(2 if i < 3 else 1))

            # initial gate: PE observes the weights DMA here, so pass-0
            # chain starts wait only on their chunk DMA
            nc.tensor.matmul(eps[:, 0:4], wm(0, 0), ws[:, 0:4],
                             start=True, stop=True)
            # warm-up: keep the PE busy until chunk0 lands so the clock
            # p-state is ramped when real work starts
            for _ in range(NDUMMY_PRE):
                nc.tensor.matmul(eps[:], wm(0, 0), ws[:, 0:S],
                                 start=True, stop=True, skip_group_check=True)

            def rhs_ap(chunk, pp, delta, moff):
                par = 1 if delta == 1 else 0
                m0 = moff + (1 if delta == 2 else 0)
                c0 = 128 * pp + 64 * par
                return chunk[:, m0:m0 + BLK, c0:c0 + 64]

            # global psum-tag rotation over 6-matmul chains; tags ps0-2 are
            # double-buffered, ps3 single.  A chain landing on a REUSED slot
            # whose chunk-DMA wait is also unobserved gets a gate matmul to
            # absorb the slot-release wait (walrus: <=1 wait/instruction).
            chain_no = 0
            alloc_count = [0, 0, 0, 0]
            bufs_of = [2, 2, 2, 1]
            group_of_chunk = {}
            for gi, grp in enumerate(OUT_GROUPS):
                for c in grp:
                    group_of_chunk[c] = gi
            osbs = {}
            last_group = len(OUT_GROUPS) - 1
            for p, (j0, w) in enumerate(zip(CHUNK_J0, CHUNK_COLS)):
                nblk = w // BLK
                chunk = chunks[p]
                gi = group_of_chunk[p]
                grp = OUT_GROUPS[gi]
                if p == grp[0]:
                    osbs[gi] = opool.tile([128, out_ws[gi], S], dt_mm,
                                          tag="osb", name=f"osb{gi}")
                osb = osbs[gi]
                loc = j0 - CHUNK_J0[grp[0]]
                pss = []
                for blk in range(nblk):
                    tag = chain_no % 4
                    reuse = alloc_count[tag] >= bufs_of[tag]
                    alloc_count[tag] += 1
                    pt = ps_tile(tag)
                    if reuse and blk == 0:
                        # gate: absorbs the psum slot-release wait so the
                        # chain start waits only on the chunk DMA
                        nc.tensor.matmul(pt[:, 0, 0:4], wm(0, 0), ws[:, 0:4],
                                         start=True, stop=True)
                    pss.append(pt)
                    chain_no += 1
                for blk in range(nblk):
                    for i in range(NW):
                        pp, delta = divmod(i, 3)
                        nc.tensor.matmul(
                            pss[blk][:], wm(pp, delta),
                            rhs_ap(chunk, pp, delta, blk * BLK),
                            start=(i == 0), stop=(i == NW - 1),
                            skip_group_check=True)
                for blk in range(nblk):
                    nc.vector.tensor_copy(
                        osb[:, loc + blk * BLK: loc + (blk + 1) * BLK, :],
                        pss[blk][:])
                if p == grp[-1]:
                    if gi == last_group:
                        nc.vector.tensor_copy(osb[:, out_ws[gi] - 1, :],
                                              edge_sb[:])
                    nc.gpsimd.dma_start(out=out_ds[gi][:], in_=osb[:])
                if p == 0:
                    # edge chain fills the PE gap while chunk1 streams in
                    for kk in range(KK):
                        nc.tensor.matmul(eps[:], we(kk), et(kk),
                                         start=(kk == 0), stop=(kk == KK - 1),
                                         skip_group_check=True)
                    nc.vector.tensor_copy(edge_sb[:], eps[:])
                    # more warm-up while the rest of chunk1 streams in
                    for _ in range(NDUMMY_POST):
                        nc.tensor.matmul(eps[:], wm(0, 0), ws[:, 0:S],
                                         start=True, stop=True,
                                         skip_group_check=True)
    return nc


# --------------------------------------------------------------- host paths
def _host_prep(x):
    """x: (32, 16, T) float32 -> per-core input maps."""
    import concourse.mybir as mybir
    np_mm = mybir.dt.np(getattr(mybir.dt, DT_MM))
    W6, W_edge = _prep_static()
    xs = np.asarray(x, np.float32).reshape(NCORES * S, T)
    left = 2.0 * xs[:, :1] - xs[:, EDGE:0:-1]
    right = 2.0 * xs[:, -1:] - xs[:, -2:-(EDGE + 2):-1]
    ext = np.concatenate([left, xs, right], axis=1)          # (512, L0)
    u = np.empty((NCORES * S, ULEN), np.float32)
    u[:, :LPAD] = ext[:, :1]
    u[:, LPAD:LPAD + L0] = ext
    u[:, LPAD + L0:] = ext[:, -1:]
    wmain = np.ascontiguousarray(W6.reshape(128, NW * 128), np_mm)
    wedge = W_edge.reshape(128, KK * 128)
    in_maps = []
    for c in range(NCORES):
        uc = u[c * S:(c + 1) * S]                            # (64, ULEN)
        # V[64*r2 + a, m2, 128*pp + 64*par + s] = u_phase[2pp+r2][s, 128*m2 + 64*par + a]
        V = np.empty((128, M2, 256), np_mm)
        for r in range(NPH):
            pp, r2 = divmod(r, 2)
            ur = np.ascontiguousarray(uc[:, r::4]).reshape(S, M2, 2, 64)
            V[64 * r2:64 * r2 + 64, :, 128 * pp:128 * pp + 128] = \
                ur.transpose(3, 1, 2, 0).reshape(64, M2, 128)
        etc = ext[c * S:(c + 1) * S, -W_EDGE:]               # (64, 1024)
        etail = np.ascontiguousarray(
            etc.T.reshape(KK, 128, S).transpose(1, 0, 2))    # [q, kk, s]
        cedge = np.concatenate(
            [wedge, etail.reshape(128, KK * S)], axis=1)     # [128, 1536]
        in_maps.append({"v": V, "wmain": wmain,
                        "cedge": np.ascontiguousarray(cedge, np_mm)})
    return in_maps


def _host_post(results):
    ys = []
    for c in range(NCORES):
        o = np.concatenate(
            [np.asarray(results[c][f"out{i}"]) for i in range(len(OUT_GROUPS))],
            axis=1).astype(np.float32)                       # [128 b, 128 j, 64 s]
        ys.append(np.ascontiguousarray(o.transpose(2, 1, 0)).reshape(S, NOUT))
    return np.concatenate(ys, axis=0).reshape(32, 16, NOUT).astype(np.float32)


def _get_nc():
    if "nc" not in _CACHE:
        _CACHE["nc"] = _build_nc()
    return _CACHE["nc"]


def kernel(x, _trace=False, _trace_kwargs=None):
    from concourse.bass_utils import run_bass_kernel_spmd
    nc = _get_nc()
    in_maps = _host_prep(x)
    res = run_bass_kernel_spmd(nc, in_maps, list(range(NCORES)),
                               trace=_trace, **(_trace_kwargs or {}))
    out = _host_post(res.results)
    if _trace:
        _CACHE["last_exec_time_ns"] = res.exec_time_ns
        _CACHE["last_result"] = res
    return out


# revision 26
# speedup vs baseline: 1.0473x; 1.0473x over previous
"""Trainium2 kernel for BandDecimate: scipy.signal.decimate(x, q=4, n=8,
ftype='iir', zero_phase=True) on x of shape (32, 16, 65536).

Method: filtfilt with the order-8 Chebyshev-I filter is re-expressed as a
single symmetric FIR g = corr(h, h) (h = 512-tap truncated impulse
response), truncated to its central 256 taps (tail l2 = 1e-3), applied to
the odd-extended, steady-state-padded signal, fused with the decimation
by 4 via a 4-phase polyphase decomposition.  Phases are PAIRED along the
PE contraction dim (2 phases x 64 samples per 128-row column), so each
128-output block needs only 6 block-Toeplitz matmuls (2 phase-pairs x 3
column offsets).  Data, weights and outputs move in bf16 (PSUM
accumulates fp32).  The right edge (last 128 decimated outputs), where
the backward-pass initial condition differs from the symmetric-FIR
approximation, is computed exactly by a dense precomputed 128x1024
linear map (8 extra matmuls).  512 independent series are sharded
64-per-core across 8 cores.
"""
import os
import sys

import numpy as np

sys.path.insert(0, "/opt/trn_rl_repo")

# ---------------------------------------------------------------- constants
Q = 4
N_ORDER = 8
RP = 0.05
T = 65536
EDGE = 27
L0 = T + 2 * EDGE          # 65590
P = 512                    # truncated IIR impulse response length
NPH = 4                    # polyphase phases
GOFF = 384                 # central-window offset into the 1024-tap G
KTAP = 256                 # total taps kept (64 per phase)
LPAD = 100                 # left steady-state pad of u (484 - GOFF)
ULEN = 66048               # 256 * 258; RPAD = ULEN - LPAD - L0 = 358
NOUT = T // Q              # 16384 outputs per series
M2 = 129                   # V columns (pairs of 64-sample half-blocks)
S = 64                     # series per core
NCORES = 8
W_EDGE = 1024              # edge window length (8 * 128)
KK = W_EDGE // 128         # 8
NW = 6                     # main weight matrices (2 phase-pairs x 3 offsets)
BLK = 8                    # output columns per matmul (free = BLK*S = 512)
PASS_COLS = (32, 32, 32, 24, 8)   # output n-columns per pass (128 total)
PASS_J0 = (0, 32, 64, 96, 120)

DT_MM = os.environ.get("BASS_MM_DTYPE", "bfloat16")


# ------------------------------------------------------------- filter design
def _design():
    eps = np.sqrt(10.0 ** (0.1 * RP) - 1.0)
    mu = np.arcsinh(1.0 / eps) / N_ORDER
    k = np.arange(1, N_ORDER + 1)
    theta = np.pi * (2 * k - 1) / (2 * N_ORDER)
    p = -np.sinh(mu) * np.sin(theta) + 1j * np.cosh(mu) * np.cos(theta)
    g = np.prod(-p).real
    if N_ORDER % 2 == 0:
        g /= np.sqrt(1.0 + eps**2)
    fs = 2.0
    warped = 2.0 * fs * np.tan(np.pi * (0.8 / Q) / fs)
    p = p * warped
    g = g * warped**N_ORDER
    fs2 = 2.0 * fs
    pd = (fs2 + p) / (fs2 - p)
    zd = -np.ones(N_ORDER)
    gd = g * np.real(1.0 / np.prod(fs2 - p))
    b = np.real(gd * np.poly(zd))
    a = np.real(np.poly(pd))
    n = len(a)
    comp = np.zeros((n - 1, n - 1))
    comp[0, :] = -a[1:] / a[0]
    comp[1:, :-1] = np.eye(n - 2)
    IminusA = np.eye(n - 1) - comp.T
    B = b[1:] - a[1:] * b[0]
    zi = np.linalg.solve(IminusA, B)
    return b, a, zi


def _lfilter(b, a, x, zi):
    """Direct-form II transposed; x: (T, M) float64."""
    z = zi.copy()
    y = np.empty_like(x)
    for t in range(x.shape[0]):
        xt = x[t]
        yt = b[0] * xt + z[0]
        y[t] = yt
        z = np.concatenate([z[1:], np.zeros_like(z[:1])], axis=0) \
            + b[1:, None] * xt[None, :] - a[1:, None] * yt[None, :]
    return y


def _build_weights():
    """Returns (W6 [128, 6, 128], W_edge [128, KK, 128]) float64."""
    b, a, zi = _design()
    x = np.zeros((P, 1))
    x[0, 0] = 1.0
    h = _lfilter(b, a, x, np.zeros((N_ORDER, 1)))[:, 0]
    g = np.correlate(h, h, mode="full")            # 1023 taps
    Gfull = np.zeros(1024)
    Gfull[: 2 * P - 1] = g
    G = Gfull[GOFF:GOFF + KTAP]                    # central 256 taps

    # main matrices: lhsT[64*r2 + a, 3*pp + delta, b] = G_{2pp+r2}[64d + a - b]
    aa = np.arange(64)[:, None]
    bb = np.arange(128)[None, :]
    W6 = np.zeros((128, NW, 128))
    for pp in range(2):
        for r2 in range(2):
            r = 2 * pp + r2
            Gr = G[r::NPH]                         # 64 taps
            for delta in range(3):
                tau = 64 * delta + aa - bb
                valid = (tau >= 0) & (tau < 64)
                W6[64 * r2:64 * r2 + 64, 3 * pp + delta, :] = \
                    np.where(valid, Gr[np.clip(tau, 0, 63)], 0.0)

    # edge matrix: exact last-128 outputs as linear map of last 1024 ext samples
    t_idx = np.arange(W_EDGE)
    w_idx = np.arange(W_EDGE)
    d_idx = t_idx[:, None] - w_idx[None, :]
    hmat = np.where((d_idx >= 0) & (d_idx < P), h[np.clip(d_idx, 0, P - 1)], 0.0)
    y1 = hmat                                      # [t, w] forward FIR basis
    y1_rev = y1[::-1]
    z0 = zi[:, None] * y1_rev[0][None, :]
    y2 = _lfilter(b, a, y1_rev, z0)[::-1]
    S0 = L0 - W_EDGE
    js = np.arange(NOUT - 128, NOUT)
    ts = EDGE + 4 * js - S0
    M_edge = y2[ts, :]                             # [128, 1024]
    W_edge = M_edge.reshape(128, KK, 128).transpose(2, 1, 0)  # [q, kk, j]
    return W6, W_edge


_CACHE = {}


def _prep_static():
    if "w" not in _CACHE:
        W6, W_edge = _build_weights()
        _CACHE["w"] = (np.ascontiguousarray(W6, np.float32),
                       np.ascontiguousarray(W_edge, np.float32))
    return _CACHE["w"]


# ------------------------------------------------------------- bass program
def _make_tile_context_cls():
    from concourse.tile import TileContext
    from concourse.vector_clock import ScopedClock, VectorClock

    class SplitDrainTileContext(TileContext):
        """This walrus build allows very few attached sync-waits per
        instruction; the stock kernel-tail drain carries one wait per DMA
        lane/engine and gets rejected.  Split it into one drain per proc,
        each with a single wait."""

        def _drain_and_barrier(self, tick_clock, wait_clock):
            gc = tick_clock.global_clock
            n = len(gc)
            for proc in range(n):
                if gc[proc] == 0:
                    continue
                vec = [0] * n
                vec[proc] = gc[proc]
                d = self.nc.sync.drain()
                wait_clock.add_sem_waits(d.ins, ScopedClock({None: VectorClock(vec)}))
            self.nc.sync.drain()
            self.nc.all_engine_barrier()
            assert self.sems is not None
            popped = self.nc._tile_sem_poison_stack.pop()
            assert popped is self._sem_poison
            self.nc.clear_and_free_semaphores(list(self.sems.allocated().values()))
            self.nc.all_engine_barrier()

    return SplitDrainTileContext


def _build_nc():
    import concourse.bass as bass
    import concourse.mybir as mybir
    TileContext = _make_tile_context_cls()

    dt_mm = getattr(mybir.dt, DT_MM)
    f32 = mybir.dt.float32

    # cedge layout along free dim: [wedge KK*128 | etail KK*64]
    CE = KK * 128 + KK * S                         # 1536
    OFF_ET = KK * 128

    nc = bass.Bass(target_bir_lowering=False)
    # v[z, m2, c]: z = 64*r2 + a, c = 128*pp + 64*par + s
    v_d = nc.declare_dram_parameter("v", [128, M2, 256], dt_mm, isOutput=False)
    # c0w = [wmain (6x128 = 3 m2-cols) | chunk0 (33 m2-cols)] in ONE tensor:
    # a single DMA delivers weights + first chunk, starting PE ~1.5us sooner
    W_M2 = NW * 128 // 256                         # 3
    c0_d = nc.declare_dram_parameter("c0w", [128, W_M2 + PASS_COLS[0] + 1, 256],
                                     dt_mm, isOutput=False)
    e_d = nc.declare_dram_parameter("cedge", [128, CE], dt_mm, isOutput=False)
    out_ds = [nc.declare_dram_parameter(f"out{i}", [128, w, S], dt_mm, isOutput=True)
              for i, w in enumerate(PASS_COLS)]

    # Walrus (this version) allows at most ONE attached sync-wait per
    # instruction, so the structure below arranges that every instruction
    # has at most one un-observed dependency:
    #  - chunk/osb pools are deep enough to never recycle a slot
    #  - ps3/eps are single-buffered; tiny "gate" matmuls absorb psum
    #    slot-release waits so chain-start matmuls wait only on their DMA
    with TileContext(nc) as tc:
        with tc.tile_pool(name="const", bufs=1) as cpool, \
             tc.tile_pool(name="vchunk", bufs=len(PASS_COLS)) as vpool, \
             tc.tile_pool(name="osb", bufs=len(PASS_COLS)) as opool, \
             tc.tile_pool(name="psum", bufs=1, space="PSUM") as ppool:

            cw = cpool.tile([128, W_M2 + PASS_COLS[0] + 1, 256], dt_mm, tag="cw")
            ce = cpool.tile([128, CE], dt_mm, tag="ce")
            edge_sb = cpool.tile([128, S], f32, tag="edge")

            # DMA issue order on the sync hw queue = transfer order:
            # weights+chunk0 as one transfer (unblocks PE), then cedge
            # (needed by the edge chain slotted into the pass0->pass1
            # gap), then the remaining chunks.
            nc.sync.dma_start(out=cw[:], in_=c0_d[:])
            nc.sync.dma_start(out=ce[:], in_=e_d[:])
            chunks = [cw]
            for p, (j0, w) in enumerate(zip(PASS_J0, PASS_COLS)):
                if p == 0:
                    continue
                chunk = vpool.tile([128, w + 1, 256], dt_mm, tag="chunk")
                nc.sync.dma_start(out=chunk[:], in_=v_d[:, j0:j0 + w + 1, :])
                chunks.append(chunk)

            def wm(pp, delta):
                k = 3 * pp + delta
                return cw[:, k // 2, (k % 2) * 128:(k % 2) * 128 + 128]

            def we(kk):
                return ce[:, kk * 128:(kk + 1) * 128]

            def et(kk):
                return ce[:, OFF_ET + kk * S: OFF_ET + (kk + 1) * S]

            eps = ppool.tile([128, S], f32, tag="eps", bufs=1)

            def ps_tile(i):
                return ppool.tile([128, BLK, S], f32, tag=f"ps{i}",
                                  name=f"ps{i}", bufs=(2 if i < 3 else 1))

            def rhs_ap(chunk, pp, delta, moff):
                par = 1 if delta == 1 else 0
                m0 = moff + (1 if delta == 2 else 0)
                c0 = 128 * pp + 64 * par
                return chunk[:, m0:m0 + BLK, c0:c0 + 64]

            npass = len(PASS_COLS)
            for p, (j0, w) in enumerate(zip(PASS_J0, PASS_COLS)):
                nblk = w // BLK
                chunk = chunks[p]
                osb = opool.tile([128, w, S], dt_mm, tag="osb")
                if p >= 2:
                    # gate: absorbs the psum slot-release (DVE) wait on PE
                    gate_tag = 3 if nblk == 1 else 0
                    gt = ps_tile(gate_tag)
                    nc.tensor.matmul(gt[:, 0, 0:4], wm(0, 0), cw[:, 0, 0:4],
                                     start=True, stop=True)
                    pss = [gt if i == 0 else ps_tile(i) for i in range(nblk)]
                else:
                    pss = [ps_tile(i) for i in range(nblk)]
                if p == 0:
                    # chunk0 lives in cw after the W_M2 weight columns
                    for blk in range(nblk):
                        for i in range(NW):
                            pp, delta = divmod(i, 3)
                            nc.tensor.matmul(
                                pss[blk][:], wm(pp, delta),
                                rhs_ap(chunk, pp, delta, W_M2 + blk * BLK),
                                start=(i == 0), stop=(i == NW - 1),
                                skip_group_check=True)
                else:
                    for i in range(NW):
                        pp, delta = divmod(i, 3)
                        for blk in range(nblk):
                            nc.tensor.matmul(
                                pss[blk][:], wm(pp, delta),
                                rhs_ap(chunk, pp, delta, blk * BLK),
                                start=(i == 0), stop=(i == NW - 1),
                                skip_group_check=True)
                # reversed cast order: ps3 (single-buffered) is released
                # first, just in time for the next pass's 4th matmul
                if p == npass - 1:
                    # edge column is ready since pass 0 -- write it BEFORE
                    # the final cast so only a 7-col cast gates the out DMA
                    nc.vector.tensor_copy(osb[:, w - 1, :], edge_sb[:])
                    nc.vector.tensor_copy(osb[:, 0:w - 1, :],
                                          pss[0][:, 0:w - 1, :])
                else:
                    for blk in reversed(range(nblk)):
                        nc.vector.tensor_copy(
                            osb[:, blk * BLK: blk * BLK + BLK, :], pss[blk][:])
                nc.gpsimd.dma_start(out=out_ds[p][:], in_=osb[:])
                if p == 0:
                    # edge chain fills the PE gap while chunk1 streams in
                    for kk in range(KK):
                        nc.tensor.matmul(eps[:], we(kk), et(kk),
                                         start=(kk == 0), stop=(kk == KK - 1),
                                         skip_group_check=True)
                    nc.vector.tensor_copy(edge_sb[:], eps[:])
    return nc


# --------------------------------------------------------------- host paths
def _host_prep(x):
    """x: (32, 16, T) float32 -> per-core input maps."""
    import concourse.mybir as mybir
    np_mm = mybir.dt.np(getattr(mybir.dt, DT_MM))
    W6, W_edge = _prep_static()
    xs = np.asarray(x, np.float32).reshape(NCORES * S, T)
    left = 2.0 * xs[:, :1] - xs[:, EDGE:0:-1]
    right = 2.0 * xs[:, -1:] - xs[:, -2:-(EDGE + 2):-1]
    ext = np.concatenate([left, xs, right], axis=1)          # (512, L0)
    u = np.empty((NCORES * S, ULEN), np.float32)
    u[:, :LPAD] = ext[:, :1]
    u[:, LPAD:LPAD + L0] = ext
    u[:, LPAD + L0:] = ext[:, -1:]
    wmain3 = np.ascontiguousarray(W6.reshape(128, NW * 128), np_mm) \
        .reshape(128, NW * 128 // 256, 256)                  # [128, 3, 256]
    wedge = W_edge.reshape(128, KK * 128)
    in_maps = []
    for c in range(NCORES):
        uc = u[c * S:(c + 1) * S]                            # (64, ULEN)
        # V[64*r2 + a, m2, 128*pp + 64*par + s] = u_phase[2pp+r2][s, 128*m2 + 64*par + a]
        V = np.empty((128, M2, 256), np_mm)
        for r in range(NPH):
            pp, r2 = divmod(r, 2)
            ur = np.ascontiguousarray(uc[:, r::4]).reshape(S, M2, 2, 64)
            V[64 * r2:64 * r2 + 64, :, 128 * pp:128 * pp + 128] = \
                ur.transpose(3, 1, 2, 0).reshape(64, M2, 128)
        etc = ext[c * S:(c + 1) * S, -W_EDGE:]               # (64, 1024)
        etail = np.ascontiguousarray(
            etc.T.reshape(KK, 128, S).transpose(1, 0, 2))    # [q, kk, s]
        cedge = np.concatenate(
            [wedge, etail.reshape(128, KK * S)], axis=1)     # [128, 1536]
        c0w = np.ascontiguousarray(np.concatenate(
            [wmain3, V[:, :PASS_COLS[0] + 1, :]], axis=1))   # [128, 36, 256]
        in_maps.append({"v": V, "c0w": c0w,
                        "cedge": np.ascontiguousarray(cedge, np_mm)})
    return in_maps


def _host_post(results):
    ys = []
    for c in range(NCORES):
        o = np.concatenate(
            [np.asarray(results[c][f"out{i}"]) for i in range(len(PASS_COLS))],
            axis=1).astype(np.float32)                       # [128 b, 128 j, 64 s]
        ys.append(np.ascontiguousarray(o.transpose(2, 1, 0)).reshape(S, NOUT))
    return np.concatenate(ys, axis=0).reshape(32, 16, NOUT).astype(np.float32)


def _get_nc():
    if "nc" not in _CACHE:
        _CACHE["nc"] = _build_nc()
    return _CACHE["nc"]


def kernel(x, _trace=False, _trace_kwargs=None):
    from concourse.bass_utils import run_bass_kernel_spmd
    nc = _get_nc()
    in_maps = _host_prep(x)
    res = run_bass_kernel_spmd(nc, in_maps, list(range(NCORES)),
                               trace=_trace, **(_trace_kwargs or {}))
    out = _host_post(res.results)
    if _trace:
        _CACHE["last_exec_time_ns"] = res.exec_time_ns
        _CACHE["last_result"] = res
    return out


# revision 33
# speedup vs baseline: 1.0998x; 1.0501x over previous
"""Trainium2 kernel for BandDecimate: scipy.signal.decimate(x, q=4, n=8,
ftype='iir', zero_phase=True) on x of shape (32, 16, 65536).

Method: filtfilt with the order-8 Chebyshev-I filter is re-expressed as a
single symmetric FIR g = corr(h, h) (h = 512-tap truncated impulse
response), truncated to its central 256 taps (tail l2 = 1e-3), applied to
the odd-extended, steady-state-padded signal, fused with the decimation
by 4 via a 4-phase polyphase decomposition.  Phases are PAIRED along the
PE contraction dim (2 phases x 64 samples per 128-row column), so each
128-output block needs only 6 block-Toeplitz matmuls (2 phase-pairs x 3
column offsets).  Data, weights and outputs move in bf16 (PSUM
accumulates fp32).  The right edge (last 128 decimated outputs), where
the backward-pass initial condition differs from the symmetric-FIR
approximation, is computed exactly by a dense precomputed 128x1024
linear map (8 extra matmuls).  512 independent series are sharded
64-per-core across 8 cores.
"""
import os
import sys

import numpy as np

sys.path.insert(0, "/opt/trn_rl_repo")

# ---------------------------------------------------------------- constants
Q = 4
N_ORDER = 8
RP = 0.05
T = 65536
EDGE = 27
L0 = T + 2 * EDGE          # 65590
P = 512                    # truncated IIR impulse response length
NPH = 4                    # polyphase phases
GOFF = 384                 # central-window offset into the 1024-tap G
KTAP = 256                 # total taps kept (64 per phase)
LPAD = 100                 # left steady-state pad of u (484 - GOFF)
ULEN = 66048               # 256 * 258; RPAD = ULEN - LPAD - L0 = 358
NOUT = T // Q              # 16384 outputs per series
M2 = 129                   # V columns (pairs of 64-sample half-blocks)
S = 64                     # series per core
NCORES = 8
W_EDGE = 1024              # edge window length (8 * 128)
KK = W_EDGE // 128         # 8
NW = 6                     # main weight matrices (2 phase-pairs x 3 offsets)
BLK = 8                    # output columns per matmul (free = BLK*S = 512)
PASS_COLS = (32, 32, 32, 24, 8)   # output n-columns per pass (128 total)
PASS_J0 = (0, 32, 64, 96, 120)

DT_MM = os.environ.get("BASS_MM_DTYPE", "bfloat16")


# ------------------------------------------------------------- filter design
def _design():
    eps = np.sqrt(10.0 ** (0.1 * RP) - 1.0)
    mu = np.arcsinh(1.0 / eps) / N_ORDER
    k = np.arange(1, N_ORDER + 1)
    theta = np.pi * (2 * k - 1) / (2 * N_ORDER)
    p = -np.sinh(mu) * np.sin(theta) + 1j * np.cosh(mu) * np.cos(theta)
    g = np.prod(-p).real
    if N_ORDER % 2 == 0:
        g /= np.sqrt(1.0 + eps**2)
    fs = 2.0
    warped = 2.0 * fs * np.tan(np.pi * (0.8 / Q) / fs)
    p = p * warped
    g = g * warped**N_ORDER
    fs2 = 2.0 * fs
    pd = (fs2 + p) / (fs2 - p)
    zd = -np.ones(N_ORDER)
    gd = g * np.real(1.0 / np.prod(fs2 - p))
    b = np.real(gd * np.poly(zd))
    a = np.real(np.poly(pd))
    n = len(a)
    comp = np.zeros((n - 1, n - 1))
    comp[0, :] = -a[1:] / a[0]
    comp[1:, :-1] = np.eye(n - 2)
    IminusA = np.eye(n - 1) - comp.T
    B = b[1:] - a[1:] * b[0]
    zi = np.linalg.solve(IminusA, B)
    return b, a, zi


def _lfilter(b, a, x, zi):
    """Direct-form II transposed; x: (T, M) float64."""
    z = zi.copy()
    y = np.empty_like(x)
    for t in range(x.shape[0]):
        xt = x[t]
        yt = b[0] * xt + z[0]
        y[t] = yt
        z = np.concatenate([z[1:], np.zeros_like(z[:1])], axis=0) \
            + b[1:, None] * xt[None, :] - a[1:, None] * yt[None, :]
    return y


def _build_weights():
    """Returns (W6 [128, 6, 128], W_edge [128, KK, 128]) float64."""
    b, a, zi = _design()
    x = np.zeros((P, 1))
    x[0, 0] = 1.0
    h = _lfilter(b, a, x, np.zeros((N_ORDER, 1)))[:, 0]
    g = np.correlate(h, h, mode="full")            # 1023 taps
    Gfull = np.zeros(1024)
    Gfull[: 2 * P - 1] = g
    G = Gfull[GOFF:GOFF + KTAP]                    # central 256 taps

    # main matrices: lhsT[64*r2 + a, 3*pp + delta, b] = G_{2pp+r2}[64d + a - b]
    aa = np.arange(64)[:, None]
    bb = np.arange(128)[None, :]
    W6 = np.zeros((128, NW, 128))
    for pp in range(2):
        for r2 in range(2):
            r = 2 * pp + r2
            Gr = G[r::NPH]                         # 64 taps
            for delta in range(3):
                tau = 64 * delta + aa - bb
                valid = (tau >= 0) & (tau < 64)
                W6[64 * r2:64 * r2 + 64, 3 * pp + delta, :] = \
                    np.where(valid, Gr[np.clip(tau, 0, 63)], 0.0)

    # edge matrix: exact last-128 outputs as linear map of last 1024 ext samples
    t_idx = np.arange(W_EDGE)
    w_idx = np.arange(W_EDGE)
    d_idx = t_idx[:, None] - w_idx[None, :]
    hmat = np.where((d_idx >= 0) & (d_idx < P), h[np.clip(d_idx, 0, P - 1)], 0.0)
    y1 = hmat                                      # [t, w] forward FIR basis
    y1_rev = y1[::-1]
    z0 = zi[:, None] * y1_rev[0][None, :]
    y2 = _lfilter(b, a, y1_rev, z0)[::-1]
    S0 = L0 - W_EDGE
    js = np.arange(NOUT - 128, NOUT)
    ts = EDGE + 4 * js - S0
    M_edge = y2[ts, :]                             # [128, 1024]
    W_edge = M_edge.reshape(128, KK, 128).transpose(2, 1, 0)  # [q, kk, j]
    return W6, W_edge


_CACHE = {}


def _prep_static():
    if "w" not in _CACHE:
        W6, W_edge = _build_weights()
        _CACHE["w"] = (np.ascontiguousarray(W6, np.float32),
                       np.ascontiguousarray(W_edge, np.float32))
    return _CACHE["w"]


# ------------------------------------------------------------- bass program
def _make_tile_context_cls():
    from concourse.tile import TileContext
    from concourse.vector_clock import ScopedClock, VectorClock

    class SplitDrainTileContext(TileContext):
        """This walrus build allows very few attached sync-waits per
        instruction; the stock kernel-tail drain carries one wait per DMA
        lane/engine and gets rejected.  Split it into one drain per proc,
        each with a single wait."""

        def _drain_and_barrier(self, tick_clock, wait_clock):
            gc = tick_clock.global_clock
            n = len(gc)
            for proc in range(n):
                if gc[proc] == 0:
                    continue
                vec = [0] * n
                vec[proc] = gc[proc]
                d = self.nc.sync.drain()
                wait_clock.add_sem_waits(d.ins, ScopedClock({None: VectorClock(vec)}))
            self.nc.sync.drain()
            self.nc.all_engine_barrier()
            assert self.sems is not None
            popped = self.nc._tile_sem_poison_stack.pop()
            assert popped is self._sem_poison
            self.nc.clear_and_free_semaphores(list(self.sems.allocated().values()))
            self.nc.all_engine_barrier()

    return SplitDrainTileContext


def _build_nc():
    import concourse.bass as bass
    import concourse.mybir as mybir
    TileContext = _make_tile_context_cls()

    dt_mm = getattr(mybir.dt, DT_MM)
    f32 = mybir.dt.float32

    # cedge layout along free dim: [wedge KK*128 | etail KK*64]
    CE = KK * 128 + KK * S                         # 1536
    OFF_ET = KK * 128

    nc = bass.Bass(target_bir_lowering=False)
    # v[z, m2, c]: z = 64*r2 + a, c = 128*pp + 64*par + s
    v_d = nc.declare_dram_parameter("v", [128, M2, 256], dt_mm, isOutput=False)
    w_d = nc.declare_dram_parameter("wmain", [128, NW * 128], dt_mm, isOutput=False)
    e_d = nc.declare_dram_parameter("cedge", [128, CE], dt_mm, isOutput=False)
    out_ds = [nc.declare_dram_parameter(f"out{i}", [128, w, S], dt_mm, isOutput=True)
              for i, w in enumerate(PASS_COLS)]

    # Walrus (this version) allows at most ONE attached sync-wait per
    # instruction, so the structure below arranges that every instruction
    # has at most one un-observed dependency:
    #  - chunk/osb pools are deep enough to never recycle a slot
    #  - ps3/eps are single-buffered; tiny "gate" matmuls absorb psum
    #    slot-release waits so chain-start matmuls wait only on their DMA
    with TileContext(nc) as tc:
        with tc.tile_pool(name="const", bufs=1) as cpool, \
             tc.tile_pool(name="vchunk", bufs=len(PASS_COLS)) as vpool, \
             tc.tile_pool(name="osb", bufs=len(PASS_COLS)) as opool, \
             tc.tile_pool(name="psum", bufs=1, space="PSUM") as ppool:

            ws = cpool.tile([128, NW * 128], dt_mm, tag="ws")
            ce = cpool.tile([128, CE], dt_mm, tag="ce")
            edge_sb = cpool.tile([128, S], f32, tag="edge")

            # DMA issue order on the sync hw queue = transfer order:
            # weights first (unblocks PE), then chunk0, cedge (needed by the
            # edge chain slotted into the pass0->pass1 gap), then the rest.
            nc.sync.dma_start(out=ws[:], in_=w_d[:])
            chunks = []
            for p, (j0, w) in enumerate(zip(PASS_J0, PASS_COLS)):
                chunk = vpool.tile([128, w + 1, 256], dt_mm, tag="chunk")
                nc.sync.dma_start(out=chunk[:], in_=v_d[:, j0:j0 + w + 1, :])
                chunks.append(chunk)
                if p == 0:
                    nc.sync.dma_start(out=ce[:], in_=e_d[:])

            def wm(pp, delta):
                k = 3 * pp + delta
                return ws[:, k * 128:(k + 1) * 128]

            def we(kk):
                return ce[:, kk * 128:(kk + 1) * 128]

            def et(kk):
                return ce[:, OFF_ET + kk * S: OFF_ET + (kk + 1) * S]

            eps = ppool.tile([128, S], f32, tag="eps", bufs=1)

            def ps_tile(i):
                return ppool.tile([128, BLK, S], f32, tag=f"ps{i}",
                                  name=f"ps{i}", bufs=(2 if i < 3 else 1))

            # initial gate: PE observes the weights DMA here, so pass-0
            # chain starts wait only on their chunk DMA
            nc.tensor.matmul(eps[:, 0:4], wm(0, 0), ws[:, 0:4],
                             start=True, stop=True)

            def rhs_ap(chunk, pp, delta, moff):
                par = 1 if delta == 1 else 0
                m0 = moff + (1 if delta == 2 else 0)
                c0 = 128 * pp + 64 * par
                return chunk[:, m0:m0 + BLK, c0:c0 + 64]

            npass = len(PASS_COLS)
            for p, (j0, w) in enumerate(zip(PASS_J0, PASS_COLS)):
                nblk = w // BLK
                chunk = chunks[p]
                osb = opool.tile([128, w, S], dt_mm, tag="osb")
                if p >= 2:
                    # gate: absorbs the psum slot-release (DVE) wait on PE
                    gate_tag = 3 if nblk == 1 else 0
                    gt = ps_tile(gate_tag)
                    nc.tensor.matmul(gt[:, 0, 0:4], wm(0, 0), ws[:, 0:4],
                                     start=True, stop=True)
                    pss = [gt if i == 0 else ps_tile(i) for i in range(nblk)]
                else:
                    pss = [ps_tile(i) for i in range(nblk)]
                if p == 0:
                    # blk-outer so compute starts as soon as the first
                    # columns of chunk0 land
                    for blk in range(nblk):
                        for i in range(NW):
                            pp, delta = divmod(i, 3)
                            nc.tensor.matmul(
                                pss[blk][:], wm(pp, delta),
                                rhs_ap(chunk, pp, delta, blk * BLK),
                                start=(i == 0), stop=(i == NW - 1),
                                skip_group_check=True)
                else:
                    for i in range(NW):
                        pp, delta = divmod(i, 3)
                        for blk in range(nblk):
                            nc.tensor.matmul(
                                pss[blk][:], wm(pp, delta),
                                rhs_ap(chunk, pp, delta, blk * BLK),
                                start=(i == 0), stop=(i == NW - 1),
                                skip_group_check=True)
                # reversed cast order: ps3 (single-buffered) is released
                # first, just in time for the next pass's 4th matmul
                for blk in reversed(range(nblk)):
                    nc.vector.tensor_copy(
                        osb[:, blk * BLK: blk * BLK + BLK, :], pss[blk][:])
                if p == npass - 1:
                    nc.vector.tensor_copy(osb[:, w - 1, :], edge_sb[:])
                nc.gpsimd.dma_start(out=out_ds[p][:], in_=osb[:])
                if p == 0:
                    # edge chain fills the PE gap while chunk1 streams in
                    for kk in range(KK):
                        nc.tensor.matmul(eps[:], we(kk), et(kk),
                                         start=(kk == 0), stop=(kk == KK - 1),
                                         skip_group_check=True)
                    nc.vector.tensor_copy(edge_sb[:], eps[:])
    return nc


# --------------------------------------------------------------- host paths
def _host_prep(x):
    """x: (32, 16, T) float32 -> per-core input maps."""
    import concourse.mybir as mybir
    np_mm = mybir.dt.np(getattr(mybir.dt, DT_MM))
    W6, W_edge = _prep_static()
    xs = np.asarray(x, np.float32).reshape(NCORES * S, T)
    left = 2.0 * xs[:, :1] - xs[:, EDGE:0:-1]
    right = 2.0 * xs[:, -1:] - xs[:, -2:-(EDGE + 2):-1]
    ext = np.concatenate([left, xs, right], axis=1)          # (512, L0)
    u = np.empty((NCORES * S, ULEN), np.float32)
    u[:, :LPAD] = ext[:, :1]
    u[:, LPAD:LPAD + L0] = ext
    u[:, LPAD + L0:] = ext[:, -1:]
    wmain = np.ascontiguousarray(W6.reshape(128, NW * 128), np_mm)
    wedge = W_edge.reshape(128, KK * 128)
    in_maps = []
    for c in range(NCORES):
        uc = u[c * S:(c + 1) * S]                            # (64, ULEN)
        # V[64*r2 + a, m2, 128*pp + 64*par + s] = u_phase[2pp+r2][s, 128*m2 + 64*par + a]
        V = np.empty((128, M2, 256), np_mm)
        for r in range(NPH):
            pp, r2 = divmod(r, 2)
            ur = np.ascontiguousarray(uc[:, r::4]).reshape(S, M2, 2, 64)
            V[64 * r2:64 * r2 + 64, :, 128 * pp:128 * pp + 128] = \
                ur.transpose(3, 1, 2, 0).reshape(64, M2, 128)
        etc = ext[c * S:(c + 1) * S, -W_EDGE:]               # (64, 1024)
        etail = np.ascontiguousarray(
            etc.T.reshape(KK, 128, S).transpose(1, 0, 2))    # [q, kk, s]
        cedge = np.concatenate(
            [wedge, etail.reshape(128, KK * S)], axis=1)     # [128, 1536]
        in_maps.append({"v": V, "wmain": wmain,
                        "cedge": np.ascontiguousarray(cedge, np_mm)})
    return in_maps


def _host_post(results):
    ys = []
    for c in range(NCORES):
        o = np.concatenate(
            [np.asarray(results[c][f"out{i}"]) for i in range(len(PASS_COLS))],
            axis=1).astype(np.float32)                       # [128 b, 128 j, 64 s]
        ys.append(np.ascontiguousarray(o.transpose(2, 1, 0)).reshape(S, NOUT))
    return np.concatenate(ys, axis=0).reshape(32, 16, NOUT).astype(np.float32)


def _get_nc():
    if "nc" not in _CACHE:
        _CACHE["nc"] = _build_nc()
    return _CACHE["nc"]


def kernel(x, _trace=False, _trace_kwargs=None):
    from concourse.bass_utils import run_bass_kernel_spmd
    nc = _get_nc()
    in_maps = _host_prep(x)
    res = run_bass_kernel_spmd(nc, in_maps, list(range(NCORES)),
                               trace=_trace, **(_trace_kwargs or {}))
    out = _host_post(res.results)
    if _trace:
        _CACHE["last_exec_time_ns"] = res.exec_time_ns
        _CACHE["last_result"] = res
    return out
